# revision 31
# baseline (speedup 1.0000x reference)
"""BiAttention TRN2 Bass kernel.

Data-parallel over batch N=8: one batch element per NeuronCore.

Per core (X = input [2048,1024], M = memory [512,1024]):
  cross  = (X @ M^T)                    (fp32r matmuls, contraction over D)
  att*32 = cross + 32*(mem_dot + NEG*(mask-1))   accumulated in PSUM
  E      = softmax over memory axis (exp on ScalarE with fused row-sum)
  inp2   = X @ W2i + b2i
  mem2   = M @ W2m + b2m
  O1     = E_norm @ mem2   (E transposed via PE, then fp32r matmul)
  w2     = softmax over LD of (in_dot + rowmax(att))  -> output_two via
           v = sum_l exp(s_l) X[l,:]  (PE), U = v @ W2i, o2 = U/Z + b2i
  out    = [inp2 | O1 | inp2*O1 | o2*O1]  (concat on free axis)

Transposes of X/M (fp32r) and E (fp32) go through the PE array (exact
pass-through); PSUM->SBUF copies round to float32r where matmul inputs
need it. float32r matmul measured at ~1.6e-4 max rel err on HW.
"""

import os
import sys

import numpy as np

for _p in ("/opt/trn_rl_repo", "/root/.axon_site/_ro/trn_rl_repo"):
    if os.path.isdir(_p) and _p not in sys.path:
        sys.path.insert(0, _p)

import concourse.bacc as bacc  # noqa: E402
import concourse.tile as tile  # noqa: E402
from concourse import bass_isa  # noqa: E402
from concourse import mybir  # noqa: E402
from concourse.bass_utils import run_bass_kernel_spmd  # noqa: E402
from concourse.masks import make_identity  # noqa: E402

P = 128
D = 1024
LD = 2048
LM = 512
HID = 1024
KC = D // P  # 8 contraction chunks
NT = LD // P  # 16 LD tiles
MC = LM // P  # 4 memory chunks
NCORES = 8
NEG = 1.0e30
RSCALE = 1.0 / 32.0  # 1/sqrt(D)

f32 = mybir.dt.float32
f32r = mybir.dt.float32r
AX = mybir.AxisListType.X
OP = mybir.AluOpType
EXP = mybir.ActivationFunctionType.Exp

TRACE = False
LAST_RESULT = None
_NC = None


def _build():
    nc = bacc.Bacc("TRN2", target_bir_lowering=False, debug=False)
    x_d = nc.dram_tensor("x", [LD, D], f32r, kind="ExternalInput")
    mem_d = nc.dram_tensor("mem", [LM, D], f32r, kind="ExternalInput")
    mask_d = nc.dram_tensor("mask", [1, LM], f32, kind="ExternalInput")
    i1w_d = nc.dram_tensor("i1w", [D], f32, kind="ExternalInput")
    m1w_d = nc.dram_tensor("m1w", [D], f32r, kind="ExternalInput")
    i2w_d = nc.dram_tensor("i2w", [D, HID], f32r, kind="ExternalInput")
    i2b_d = nc.dram_tensor("i2b", [1, HID], f32, kind="ExternalInput")
    m2w_d = nc.dram_tensor("m2w", [D, HID], f32r, kind="ExternalInput")
    m2b_d = nc.dram_tensor("m2b", [1, HID], f32, kind="ExternalInput")
    out_d = nc.dram_tensor("out", [LD, 4 * HID], f32, kind="ExternalOutput")
    mem2_d = nc.dram_tensor("mem2o", [LM, HID], f32r, kind="ExternalOutput")

    with tile.TileContext(nc) as tc:
        with (
            tc.tile_pool(name="persist", bufs=1) as pp,
            tc.tile_pool(name="psA", bufs=6, space="PSUM") as psA,
            tc.tile_pool(name="psB", bufs=2, space="PSUM") as psB,
        ):
            # ---- constants ----
            ident = pp.tile([P, P], f32, tag="ident")
            make_identity(nc, ident[:])
            ident_r = pp.tile([P, P], f32r, tag="identr")
            nc.vector.tensor_copy(ident_r[:], ident[:])
            ones_f = pp.tile([1, P], f32, tag="onesf")
            nc.gpsimd.memset(ones_f[:], 1.0)
            ones_r = pp.tile([1, P], f32r, tag="onesr")
            nc.vector.tensor_copy(ones_r[:], ones_f[:])

            # ---- persistent tensors ----
            mt = [
                pp.tile([P, LM], f32r, tag=f"mt{k}", name=f"mt{k}") for k in range(KC)
            ]
            mem2 = [
                pp.tile([P, HID], f32r, tag=f"mem2_{j}", name=f"mem2_{j}")
                for j in range(MC)
            ]
            o1k = [
                pp.tile([P, HID], f32, tag=f"o1k{t}", name=f"o1k{t}") for t in range(NT)
            ]
            G = pp.tile([P, NT], f32, tag="G")
            v2t = pp.tile([33, D], f32, tag="vsb")
            nc.gpsimd.memset(v2t[:], 0.0)
            v_sb = v2t[0:1, :]
            vb_sb = v2t[32:33, :]
            vTa_f = pp.tile([P, KC], f32, tag="vTaf")
            row_bias = pp.tile([1, LM], f32r, tag="rowbias")

            # ================= preamble =================
            with tc.tile_pool(name="pre", bufs=1) as pre:
                # transpose memory -> mt[k] = M^T chunk [128(d), 512(m)]
                for j in range(MC):
                    memt = pre.tile([P, D], f32r, tag=f"memt{j % 2}", name=f"memt{j}")
                    nc.sync.dma_start(memt[:], mem_d[j * P : (j + 1) * P, :])
                    for k in range(KC):
                        pt = psB.tile([P, P], f32r, tag="tr")
                        nc.tensor.transpose(pt[:], memt[:, k * P : (k + 1) * P], ident_r[:])
                        if k % 2 == 0:
                            nc.vector.tensor_copy(mt[k][:, j * P : (j + 1) * P], pt[:])
                        else:
                            nc.scalar.copy(mt[k][:, j * P : (j + 1) * P], pt[:])
                # ---- small weights (issued after the memory loads) ----
                m1w = pp.tile([P, KC], f32r, tag="m1w")
                nc.sync.dma_start(m1w[:], m1w_d[:].rearrange("(c p) -> p c", p=P))
                mask_row = pp.tile([1, LM], f32, tag="maskrow")
                nc.sync.dma_start(mask_row[:], mask_d[:])
                b2i_row = pp.tile([1, HID], f32, tag="b2irow")
                nc.sync.dma_start(b2i_row[:], i2b_d[:])
                b2i_row_r = pp.tile([1, HID], f32r, tag="b2irowr")
                nc.vector.tensor_copy(b2i_row_r[:], b2i_row[:])
                i1w_f = pp.tile([P, 2 * KC], f32, tag="i1wf")
                nc.gpsimd.memset(i1w_f[:], 0.0)
                nc.sync.dma_start(
                    i1w_f[:, 0 : 2 * KC : 2], i1w_d[:].rearrange("(c p) -> p c", p=P)
                )
                i1w = pp.tile([P, 2 * KC], f32r, tag="i1w")
                nc.vector.tensor_copy(i1w[:], i1w_f[:])

                # mem_dot row [1, LM]
                mdp = psA.tile([1, LM], f32, tag="A")
                for k in range(KC):
                    nc.tensor.matmul(
                        mdp[:], m1w[:, k : k + 1], mt[k][:],
                        start=(k == 0), stop=(k == KC - 1),
                    )
                # row_bias = 32*mem_dot + 32*NEG*(mask-1)
                maskt = pre.tile([1, LM], f32, tag="maskt")
                nc.vector.tensor_scalar(
                    maskt[:], mask_row[:], 32.0 * NEG, -32.0 * NEG, OP.mult, OP.add
                )
                md32 = pre.tile([1, LM], f32, tag="md32")
                nc.vector.tensor_scalar_mul(md32[:], mdp[:], 32.0)
                nc.vector.tensor_tensor(row_bias[:], md32[:], maskt[:], OP.add)


                # mem2 = M @ W2m + b2m
                b2m_row = pre.tile([1, HID], f32, tag="b2mrow")
                nc.sync.dma_start(b2m_row[:], m2b_d[:])
                b2m_row_r = pre.tile([1, HID], f32r, tag="b2mrowr")
                nc.vector.tensor_copy(b2m_row_r[:], b2m_row[:])
                # interleave W2i (tile-0 k-loop) and W2m (mem2) chunk loads so
                # both consumers start as their first chunks land
                w2i = []
                w2m = []
                for k in range(KC):
                    wi = pp.tile([P, HID], f32r, tag=f"w2i{k}", name=f"w2i{k}")
                    nc.sync.dma_start(wi[:], i2w_d[k * P : (k + 1) * P, :])
                    w2i.append(wi)
                    wm = pre.tile([P, HID], f32r, tag=f"w2m{k}", name=f"w2m{k}")
                    nc.sync.dma_start(wm[:], m2w_d[k * P : (k + 1) * P, :])
                    w2m.append(wm)
                for j in range(MC):
                    pa = psA.tile([P, 512], f32, tag="A")
                    pb = psA.tile([P, 512], f32, tag="A")
                    nc.tensor.matmul(pa[:], ones_r[:], b2m_row_r[:, 0:512], start=True, stop=False)
                    nc.tensor.matmul(pb[:], ones_r[:], b2m_row_r[:, 512:HID], start=True, stop=False)
                    for k in range(KC):
                        lhs = mt[k][:, j * P : (j + 1) * P]
                        nc.tensor.matmul(
                            pa[:], lhs, w2m[k][:, 0:512],
                            start=False, stop=(k == KC - 1),
                        )
                        nc.tensor.matmul(
                            pb[:], lhs, w2m[k][:, 512:HID],
                            start=False, stop=(k == KC - 1),
                        )
                    nc.scalar.copy(mem2[j][:, 0:512], pa[:])
                    nc.scalar.copy(mem2[j][:, 512:HID], pb[:])
                    nc.gpsimd.dma_start(mem2_d[j * P : (j + 1) * P, :], mem2[j][:])

            # ================= main loop over LD tiles =================
            with tc.tile_pool(name="wk", bufs=2) as wk, tc.tile_pool(name="wk3", bufs=3) as wk3:
                for t in range(NT):
                    rows = slice(t * P, (t + 1) * P)
                    xnat = wk.tile([P, D], f32r, tag="xnat")
                    nc.gpsimd.dma_start(xnat[:], x_d[rows, :])
                    # X^T chunks: xt[:, k*P:(k+1)*P] = [128(d of chunk k), 128(l)]
                    xt = wk3.tile([P, D], f32r, tag="xt")
                    for k in range(KC):
                        pt = psB.tile([P, P], f32r, tag="tr")
                        nc.tensor.transpose(pt[:], xnat[:, k * P : (k + 1) * P], ident_r[:])
                        if k % 2 == 0:
                            nc.vector.tensor_copy(xt[:, k * P : (k + 1) * P], pt[:])
                        else:
                            nc.scalar.copy(xt[:, k * P : (k + 1) * P], pt[:])

                    cr = psA.tile([P, LM], f32, tag="A")
                    ia = psA.tile([P, 512], f32, tag="A")
                    ib = psA.tile([P, 512], f32, tag="A")
                    idp = psA.tile([P, 2], f32, tag="A")
                    # bias row first, then cross chunks; the group closes at the
                    # last cross matmul so softmax starts as early as possible
                    nc.tensor.matmul(cr[:], ones_r[:], row_bias[:], start=True, stop=False)
                    nc.tensor.matmul(ia[:], ones_r[:], b2i_row_r[:, 0:512], start=True, stop=False)
                    nc.tensor.matmul(ib[:], ones_r[:], b2i_row_r[:, 512:HID], start=True, stop=False)
                    for k in range(KC):
                        nc.tensor.matmul(
                            cr[:], xt[:, k * P : (k + 1) * P], mt[k][:],
                            start=False, stop=(k == KC - 1),
                        )
                    for k in range(KC):
                        lhs = xt[:, k * P : (k + 1) * P]
                        nc.tensor.matmul(
                            ia[:], lhs, w2i[k][:, 0:512],
                            start=False, stop=(k == KC - 1),
                        )
                        nc.tensor.matmul(
                            ib[:], lhs, w2i[k][:, 512:HID],
                            start=False, stop=(k == KC - 1),
                        )
                        nc.tensor.matmul(
                            idp[:], lhs, i1w[:, 2 * k : 2 * k + 2],
                            start=(k == 0), stop=(k == KC - 1),
                        )

                    # softmax over the memory axis. att values are O(10), so
                    # exp() cannot overflow: skip the max-subtraction for E and
                    # keep rowmax only for the weight_two statistic (it is part
                    # of the math there, not just stabilization). This takes the
                    # DVE reduce off the cr->E critical path.
                    E = wk3.tile([P, LM], f32r, tag="E")
                    dsum = wk.tile([P, 1], f32, tag="dsum")
                    nc.scalar.activation(
                        E[:], cr[:], EXP, scale=RSCALE, accum_out=dsum[:]
                    )
                    nmx = wk.tile([P, 1], f32, tag="nmx")
                    nc.vector.reduce_max(nmx[:], cr[:], axis=AX, negate=True)
                    nbias = wk.tile([P, 1], f32, tag="nbias")
                    nc.vector.tensor_scalar_mul(nbias[:], nmx[:], RSCALE)
                    rc = wk.tile([P, 1], f32, tag="rc")
                    nc.vector.reciprocal(rc[:], dsum[:])
                    # s = in_dot + rowmax/32 ; exp(s) -> G column t
                    s_t = wk.tile([P, 1], f32, tag="s")
                    nc.vector.tensor_tensor(s_t[:], idp[:, 0:1], nbias[:], OP.subtract)
                    nc.scalar.activation(G[:, t : t + 1], s_t[:], EXP)
                    es_r = wk.tile([P, 1], f32r, tag="esr")
                    nc.vector.tensor_copy(es_r[:], G[:, t : t + 1])

                    # v += exp(s)^T @ X   (for output_two)
                    vtgt = v_sb if t < 12 else vb_sb
                    v1 = psA.tile([1, 512], f32, tag="A")
                    nc.tensor.matmul(v1[:], es_r[:], xnat[:, 0:512], start=True, stop=True)
                    nc.vector.tensor_tensor(vtgt[:, 0:512], vtgt[:, 0:512], v1[:], OP.add)
                    v2 = psA.tile([1, 512], f32, tag="A")
                    nc.tensor.matmul(v2[:], es_r[:], xnat[:, 512:D], start=True, stop=True)
                    nc.vector.tensor_tensor(vtgt[:, 512:D], vtgt[:, 512:D], v2[:], OP.add)
                    if t == 11:
                        # bounce the tiles-0..11 partial through DRAM early so its
                        # round-trip hides under the remaining tiles
                        with tc.tile_pool(name="dscA", bufs=1, space="DRAM") as dscA:
                            va_dram = dscA.tile([1, D], f32, name="va_dram")
                            nc.sync.dma_start(va_dram[:], v_sb[:])
                            nc.sync.dma_start(
                                vTa_f[:], va_dram[:].rearrange("1 (c p) -> p c", p=P)
                            )

                    # inp2 tile -> out[:, 0:1024]
                    inp2 = wk.tile([P, HID], f32, tag="inp2")
                    nc.scalar.copy(inp2[:, 0:512], ia[:])
                    nc.scalar.copy(inp2[:, 512:HID], ib[:])
                    nc.sync.dma_start(out_d[rows, 0:HID], inp2[:])

                    # E^T chunks
                    et = wk3.tile([P, LM], f32r, tag="et")
                    for j in range(MC):
                        pt = psB.tile([P, P], f32r, tag="tr")
                        nc.tensor.transpose(pt[:], E[:, j * P : (j + 1) * P], ident_r[:])
                        if j % 2 == 0:
                            nc.vector.tensor_copy(et[:, j * P : (j + 1) * P], pt[:])
                        else:
                            nc.scalar.copy(et[:, j * P : (j + 1) * P], pt[:])

                    # O1 = E_norm @ mem2 -> out[:, 1024:2048] (scaled by 1/denom on copy)
                    o1a = psA.tile([P, 512], f32, tag="A")
                    o1b = psA.tile([P, 512], f32, tag="A")
                    for j in range(MC):
                        lhs = et[:, j * P : (j + 1) * P]
                        nc.tensor.matmul(
                            o1a[:], lhs, mem2[j][:, 0:512],
                            start=(j == 0), stop=(j == MC - 1),
                        )
                        nc.tensor.matmul(
                            o1b[:], lhs, mem2[j][:, 512:HID],
                            start=(j == 0), stop=(j == MC - 1),
                        )
                    nc.scalar.mul(o1k[t][:, 0:512], o1a[:], rc[:])
                    nc.scalar.mul(o1k[t][:, 512:HID], o1b[:], rc[:])
                    nc.sync.dma_start(out_d[rows, HID : 2 * HID], o1k[t][:])

                    # out3 = inp2 * O1 -> out[:, 2048:3072]
                    out3 = wk.tile([P, HID], f32, tag="out3")
                    nc.vector.tensor_tensor(
                        out3[:, 0:512], inp2[:, 0:512], o1k[t][:, 0:512], OP.mult
                    )
                    nc.vector.tensor_tensor(
                        out3[:, 512:HID], inp2[:, 512:HID], o1k[t][:, 512:HID], OP.mult
                    )
                    nc.gpsimd.dma_start(out_d[rows, 2 * HID : 3 * HID], out3[:])

                # ============ epilogue: output_two ============
                ep_cm = tc.tile_pool(name="ep", bufs=1)
                ep = ep_cm.__enter__()
                g1 = ep.tile([P, 1], f32, tag="g1")
                nc.vector.reduce_sum(g1[:], G[:], axis=AX)
                g1r = ep.tile([P, 1], f32r, tag="g1r")
                nc.vector.tensor_copy(g1r[:], g1[:])
                ones2_f = ep.tile([P, 2], f32, tag="ones2f")
                nc.gpsimd.memset(ones2_f[:], 1.0)
                ones2_r = ep.tile([P, 2], f32r, tag="ones2r")
                nc.vector.tensor_copy(ones2_r[:], ones2_f[:])
                zps = psA.tile([1, 2], f32, tag="A")
                nc.tensor.matmul(zps[:], g1r[:], ones2_r[:], start=True, stop=True)
                rcz = ep.tile([1, 1], f32, tag="rcz")
                nc.vector.reciprocal(rcz[:], zps[0:1, 0:1])
                # v^T [128, KC]
                with tc.tile_pool(name="dsc", bufs=1, space="DRAM") as dsc:
                    v_dram = dsc.tile([1, D], f32, name="v_dram")
                    nc.sync.dma_start(v_dram[:], vb_sb[:])
                    vT_f = ep.tile([P, KC], f32, tag="vTf")
                    nc.sync.dma_start(
                        vT_f[:], v_dram[:].rearrange("1 (c p) -> p c", p=P)
                    )
                vT = ep.tile([P, KC], f32r, tag="vT")
                nc.vector.tensor_tensor(vT_f[:], vT_f[:], vTa_f[:], OP.add)
                nc.vector.tensor_copy(vT[:], vT_f[:])
                ua = psA.tile([1, 512], f32, tag="A")
                ub = psA.tile([1, 512], f32, tag="A")
                for k in range(KC):
                    nc.tensor.matmul(
                        ua[:], vT[:, k : k + 1], w2i[k][:, 0:512],
                        start=(k == 0), stop=(k == KC - 1),
                    )
                    nc.tensor.matmul(
                        ub[:], vT[:, k : k + 1], w2i[k][:, 512:HID],
                        start=(k == 0), stop=(k == KC - 1),
                    )
                o2row = ep.tile([1, HID], f32, tag="o2row")
                nc.vector.tensor_scalar_mul(o2row[:, 0:512], ua[:], rcz[:])
                nc.vector.tensor_scalar_mul(o2row[:, 512:HID], ub[:], rcz[:])
                o2row_r = ep.tile([1, HID], f32r, tag="o2rowr")
                nc.vector.tensor_tensor(o2row_r[:], o2row[:], b2i_row[:], OP.add)
                # broadcast across partitions via K=1 matmuls (PE is idle here)
                o2bc = ep.tile([P, HID], f32, tag="o2bc")
                ob1 = psA.tile([P, 512], f32, tag="A")
                nc.tensor.matmul(ob1[:], ones_r[:], o2row_r[:, 0:512], start=True, stop=True)
                nc.scalar.copy(o2bc[:, 0:512], ob1[:])
                ob2 = psA.tile([P, 512], f32, tag="A")
                nc.tensor.matmul(ob2[:], ones_r[:], o2row_r[:, 512:HID], start=True, stop=True)
                nc.scalar.copy(o2bc[:, 512:HID], ob2[:])
                # out4 = o2 * O1 -> out[:, 3072:4096]
                for t in range(NT):
                    rows = slice(t * P, (t + 1) * P)
                    out4 = wk.tile([P, HID], f32, tag="out3", name=f"out4_{t}")
                    nc.vector.tensor_tensor(
                        out4[:, 0:512], o1k[t][:, 0:512], o2bc[:, 0:512], OP.mult
                    )
                    eng4 = nc.gpsimd if t % 2 == 1 else nc.vector
                    eng4.tensor_tensor(
                        out4[:, 512:HID], o1k[t][:, 512:HID], o2bc[:, 512:HID], OP.mult
                    )
                    nc.sync.dma_start(out_d[rows, 3 * HID : 4 * HID], out4[:])
                ep_cm.__exit__(None, None, None)

    nc.compile()
    return nc


def _get_nc():
    global _NC
    if _NC is None:
        _NC = _build()
    return _NC


def kernel(input, memory, mask, in1_w, mem1_w, in2_w, in2_b, mem2_w, mem2_b):
    nc = _get_nc()

    def f(a):
        return np.ascontiguousarray(np.asarray(a, dtype=np.float32))

    inp = f(input)
    memv = f(memory)
    maskv = f(mask)
    i1 = f(in1_w)
    m1 = f(mem1_w)
    wi = f(in2_w)
    bi = f(in2_b).reshape(1, HID)
    wm = f(mem2_w)
    bm = f(mem2_b).reshape(1, HID)

    in_maps = []
    for i in range(NCORES):
        in_maps.append(
            {
                "x": inp[i],
                "mem": memv[i],
                "mask": maskv[i].reshape(1, LM),
                "i1w": i1,
                "m1w": m1,
                "i2w": wi,
                "i2b": bi,
                "m2w": wm,
                "m2b": bm,
            }
        )

    kwargs = {}
    if TRACE:
        kwargs["trace"] = True
    res = run_bass_kernel_spmd(nc, in_maps, core_ids=list(range(NCORES)), **kwargs)
    global LAST_RESULT
    LAST_RESULT = res

    out = np.stack([r["out"] for r in res.results])
    mem2 = np.stack([r["mem2o"] for r in res.results])
    return out, mem2


# revision 32
# speedup vs baseline: 1.0420x; 1.0420x over previous
"""BiAttention TRN2 Bass kernel.

Data-parallel over batch N=8: one batch element per NeuronCore.

Per core (X = input [2048,1024], M = memory [512,1024]):
  cross  = (X @ M^T)                    (fp32r matmuls, contraction over D)
  att*32 = cross + 32*(mem_dot + NEG*(mask-1))   accumulated in PSUM
  E      = softmax over memory axis (exp on ScalarE with fused row-sum)
  inp2   = X @ W2i + b2i
  mem2   = M @ W2m + b2m
  O1     = E_norm @ mem2   (E transposed via PE, then fp32r matmul)
  w2     = softmax over LD of (in_dot + rowmax(att))  -> output_two via
           v = sum_l exp(s_l) X[l,:]  (PE), U = v @ W2i, o2 = U/Z + b2i
  out    = [inp2 | O1 | inp2*O1 | o2*O1]  (concat on free axis)

Transposes of X/M (fp32r) and E (fp32) go through the PE array (exact
pass-through); PSUM->SBUF copies round to float32r where matmul inputs
need it. float32r matmul measured at ~1.6e-4 max rel err on HW.
"""

import os
import sys

import numpy as np

for _p in ("/opt/trn_rl_repo", "/root/.axon_site/_ro/trn_rl_repo"):
    if os.path.isdir(_p) and _p not in sys.path:
        sys.path.insert(0, _p)

import concourse.bacc as bacc  # noqa: E402
import concourse.tile as tile  # noqa: E402
from concourse import bass_isa  # noqa: E402
from concourse import mybir  # noqa: E402
from concourse.bass_utils import run_bass_kernel_spmd  # noqa: E402
from concourse.masks import make_identity  # noqa: E402

P = 128
D = 1024
LD = 2048
LM = 512
HID = 1024
KC = D // P  # 8 contraction chunks
NT = LD // P  # 16 LD tiles
MC = LM // P  # 4 memory chunks
NCORES = 8
NEG = 1.0e30
RSCALE = 1.0 / 32.0  # 1/sqrt(D)

f32 = mybir.dt.float32
f32r = mybir.dt.float32r
AX = mybir.AxisListType.X
OP = mybir.AluOpType
EXP = mybir.ActivationFunctionType.Exp

TRACE = False
LAST_RESULT = None
_NC = None


def _build():
    nc = bacc.Bacc("TRN2", target_bir_lowering=False, debug=False)
    x_d = nc.dram_tensor("x", [LD, D], f32r, kind="ExternalInput")
    mem_d = nc.dram_tensor("mem", [LM, D], f32r, kind="ExternalInput")
    mask_d = nc.dram_tensor("mask", [1, LM], f32, kind="ExternalInput")
    i1w_d = nc.dram_tensor("i1w", [D], f32, kind="ExternalInput")
    m1w_d = nc.dram_tensor("m1w", [D], f32r, kind="ExternalInput")
    i2w_d = nc.dram_tensor("i2w", [D, HID], f32r, kind="ExternalInput")
    i2b_d = nc.dram_tensor("i2b", [1, HID], f32, kind="ExternalInput")
    m2w_d = nc.dram_tensor("m2w", [D, HID], f32r, kind="ExternalInput")
    m2b_d = nc.dram_tensor("m2b", [1, HID], f32, kind="ExternalInput")
    out_d = nc.dram_tensor("out", [LD, 4 * HID], f32, kind="ExternalOutput")
    mem2_d = nc.dram_tensor("mem2o", [LM, HID], f32r, kind="ExternalOutput")

    with tile.TileContext(nc) as tc:
        with (
            tc.tile_pool(name="persist", bufs=1) as pp,
            tc.tile_pool(name="psA", bufs=6, space="PSUM") as psA,
            tc.tile_pool(name="psB", bufs=2, space="PSUM") as psB,
        ):
            # ---- constants ----
            ident = pp.tile([P, P], f32, tag="ident")
            make_identity(nc, ident[:])
            ident_r = pp.tile([P, P], f32r, tag="identr")
            nc.vector.tensor_copy(ident_r[:], ident[:])
            ones_f = pp.tile([1, P], f32, tag="onesf")
            nc.gpsimd.memset(ones_f[:], 1.0)
            ones_r = pp.tile([1, P], f32r, tag="onesr")
            nc.vector.tensor_copy(ones_r[:], ones_f[:])

            # ---- persistent tensors ----
            mt = [
                pp.tile([P, LM], f32r, tag=f"mt{k}", name=f"mt{k}") for k in range(KC)
            ]
            mem2 = [
                pp.tile([P, HID], f32r, tag=f"mem2_{j}", name=f"mem2_{j}")
                for j in range(MC)
            ]
            o1k = [
                pp.tile([P, HID], f32, tag=f"o1k{t}", name=f"o1k{t}") for t in range(NT)
            ]
            G = pp.tile([P, NT], f32, tag="G")
            v2t = pp.tile([33, D], f32, tag="vsb")
            nc.gpsimd.memset(v2t[:], 0.0)
            v_sb = v2t[0:1, :]
            vb_sb = v2t[32:33, :]
            vTa_f = pp.tile([P, KC], f32, tag="vTaf")
            row_bias = pp.tile([1, LM], f32r, tag="rowbias")

            # ================= preamble =================
            with tc.tile_pool(name="pre", bufs=1) as pre:
                # transpose memory -> mt[k] = M^T chunk [128(d), 512(m)]
                for j in range(MC):
                    memt = pre.tile([P, D], f32r, tag=f"memt{j % 2}", name=f"memt{j}")
                    nc.sync.dma_start(memt[:], mem_d[j * P : (j + 1) * P, :])
                    for k in range(KC):
                        pt = psB.tile([P, P], f32r, tag="tr")
                        nc.tensor.transpose(pt[:], memt[:, k * P : (k + 1) * P], ident_r[:])
                        if k % 2 == 0:
                            nc.vector.tensor_copy(mt[k][:, j * P : (j + 1) * P], pt[:])
                        else:
                            nc.scalar.copy(mt[k][:, j * P : (j + 1) * P], pt[:])
                # ---- small weights (issued after the memory loads) ----
                m1w = pp.tile([P, KC], f32r, tag="m1w")
                nc.sync.dma_start(m1w[:], m1w_d[:].rearrange("(c p) -> p c", p=P))
                mask_row = pp.tile([1, LM], f32, tag="maskrow")
                nc.sync.dma_start(mask_row[:], mask_d[:])
                b2i_row = pp.tile([1, HID], f32, tag="b2irow")
                nc.sync.dma_start(b2i_row[:], i2b_d[:])
                b2i_row_r = pp.tile([1, HID], f32r, tag="b2irowr")
                nc.vector.tensor_copy(b2i_row_r[:], b2i_row[:])
                i1w_f = pp.tile([P, 2 * KC], f32, tag="i1wf")
                nc.gpsimd.memset(i1w_f[:], 0.0)
                nc.sync.dma_start(
                    i1w_f[:, 0 : 2 * KC : 2], i1w_d[:].rearrange("(c p) -> p c", p=P)
                )
                i1w = pp.tile([P, 2 * KC], f32r, tag="i1w")
                nc.vector.tensor_copy(i1w[:], i1w_f[:])

                # mem_dot row [1, LM]
                mdp = psA.tile([1, LM], f32, tag="A")
                for k in range(KC):
                    nc.tensor.matmul(
                        mdp[:], m1w[:, k : k + 1], mt[k][:],
                        start=(k == 0), stop=(k == KC - 1),
                    )
                # row_bias = 32*mem_dot + 32*NEG*(mask-1)
                maskt = pre.tile([1, LM], f32, tag="maskt")
                nc.vector.tensor_scalar(
                    maskt[:], mask_row[:], 32.0 * NEG, -32.0 * NEG, OP.mult, OP.add
                )
                md32 = pre.tile([1, LM], f32, tag="md32")
                nc.vector.tensor_scalar_mul(md32[:], mdp[:], 32.0)
                nc.vector.tensor_tensor(row_bias[:], md32[:], maskt[:], OP.add)


                # mem2 = M @ W2m + b2m
                b2m_row = pre.tile([1, HID], f32, tag="b2mrow")
                nc.sync.dma_start(b2m_row[:], m2b_d[:])
                b2m_row_r = pre.tile([1, HID], f32r, tag="b2mrowr")
                nc.vector.tensor_copy(b2m_row_r[:], b2m_row[:])
                # interleave W2i (tile-0 k-loop) and W2m (mem2) chunk loads so
                # both consumers start as their first chunks land
                w2i = []
                w2m = []
                for k in range(KC):
                    wi = pp.tile([P, HID], f32r, tag=f"w2i{k}", name=f"w2i{k}")
                    nc.sync.dma_start(wi[:], i2w_d[k * P : (k + 1) * P, :])
                    w2i.append(wi)
                    wm = pre.tile([P, HID], f32r, tag=f"w2m{k}", name=f"w2m{k}")
                    nc.sync.dma_start(wm[:], m2w_d[k * P : (k + 1) * P, :])
                    w2m.append(wm)
                for j in range(MC):
                    pa = psA.tile([P, 512], f32, tag="A")
                    pb = psA.tile([P, 512], f32, tag="A")
                    nc.tensor.matmul(pa[:], ones_r[:], b2m_row_r[:, 0:512], start=True, stop=False)
                    nc.tensor.matmul(pb[:], ones_r[:], b2m_row_r[:, 512:HID], start=True, stop=False)
                    for k in range(KC):
                        lhs = mt[k][:, j * P : (j + 1) * P]
                        nc.tensor.matmul(
                            pa[:], lhs, w2m[k][:, 0:512],
                            start=False, stop=(k == KC - 1),
                        )
                        nc.tensor.matmul(
                            pb[:], lhs, w2m[k][:, 512:HID],
                            start=False, stop=(k == KC - 1),
                        )
                    nc.scalar.copy(mem2[j][:, 0:512], pa[:])
                    nc.scalar.copy(mem2[j][:, 512:HID], pb[:])
                    nc.gpsimd.dma_start(mem2_d[j * P : (j + 1) * P, :], mem2[j][:])

            # ================= main loop over LD tiles =================
            with tc.tile_pool(name="wk", bufs=2) as wk, tc.tile_pool(name="wk3", bufs=3) as wk3:
                for t in range(NT):
                    rows = slice(t * P, (t + 1) * P)
                    xnat = wk.tile([P, D], f32r, tag="xnat")
                    nc.gpsimd.dma_start(xnat[:], x_d[rows, :])
                    # X^T chunks: xt[:, k*P:(k+1)*P] = [128(d of chunk k), 128(l)]
                    xt = wk3.tile([P, D], f32r, tag="xt")
                    for k in range(KC):
                        pt = psB.tile([P, P], f32r, tag="tr")
                        nc.tensor.transpose(pt[:], xnat[:, k * P : (k + 1) * P], ident_r[:])
                        if k % 2 == 0:
                            nc.vector.tensor_copy(xt[:, k * P : (k + 1) * P], pt[:])
                        else:
                            nc.scalar.copy(xt[:, k * P : (k + 1) * P], pt[:])

                    cr = psA.tile([P, LM], f32, tag="A")
                    ia = psA.tile([P, 512], f32, tag="A")
                    ib = psA.tile([P, 512], f32, tag="A")
                    idp = psA.tile([P, 2], f32, tag="A")
                    # bias row first, then cross chunks; the group closes at the
                    # last cross matmul so softmax starts as early as possible
                    nc.tensor.matmul(cr[:], ones_r[:], row_bias[:], start=True, stop=False)
                    nc.tensor.matmul(ia[:], ones_r[:], b2i_row_r[:, 0:512], start=True, stop=False)
                    nc.tensor.matmul(ib[:], ones_r[:], b2i_row_r[:, 512:HID], start=True, stop=False)
                    for k in range(KC):
                        nc.tensor.matmul(
                            cr[:], xt[:, k * P : (k + 1) * P], mt[k][:],
                            start=False, stop=(k == KC - 1),
                        )
                    for k in range(KC):
                        lhs = xt[:, k * P : (k + 1) * P]
                        nc.tensor.matmul(
                            ia[:], lhs, w2i[k][:, 0:512],
                            start=False, stop=(k == KC - 1),
                        )
                        nc.tensor.matmul(
                            ib[:], lhs, w2i[k][:, 512:HID],
                            start=False, stop=(k == KC - 1),
                        )
                        nc.tensor.matmul(
                            idp[:], lhs, i1w[:, 2 * k : 2 * k + 2],
                            start=(k == 0), stop=(k == KC - 1),
                        )

                    # softmax over the memory axis. att values are O(10), so
                    # exp() cannot overflow: skip the max-subtraction for E and
                    # keep rowmax only for the weight_two statistic (it is part
                    # of the math there, not just stabilization). This takes the
                    # DVE reduce off the cr->E critical path.
                    E = wk3.tile([P, LM], f32r, tag="E")
                    dsum = wk.tile([P, 1], f32, tag="dsum")
                    nc.scalar.activation(
                        E[:], cr[:], EXP, scale=RSCALE, accum_out=dsum[:]
                    )
                    nmx = wk.tile([P, 1], f32, tag="nmx")
                    nc.vector.reduce_max(nmx[:], cr[:], axis=AX, negate=True)
                    nbias = wk.tile([P, 1], f32, tag="nbias")
                    nc.vector.tensor_scalar_mul(nbias[:], nmx[:], RSCALE)
                    rc = wk.tile([P, 1], f32, tag="rc")
                    nc.vector.reciprocal(rc[:], dsum[:])
                    # s = in_dot + rowmax/32 ; exp(s) -> G column t
                    s_t = wk.tile([P, 1], f32, tag="s")
                    nc.vector.tensor_tensor(s_t[:], idp[:, 0:1], nbias[:], OP.subtract)
                    nc.scalar.activation(G[:, t : t + 1], s_t[:], EXP)
                    es_r = wk.tile([P, 1], f32r, tag="esr")
                    nc.vector.tensor_copy(es_r[:], G[:, t : t + 1])

                    # v += exp(s)^T @ X   (for output_two)
                    vtgt = v_sb if t < 12 else vb_sb
                    v1 = psA.tile([1, 512], f32, tag="A")
                    nc.tensor.matmul(v1[:], es_r[:], xnat[:, 0:512], start=True, stop=True)
                    nc.vector.tensor_tensor(vtgt[:, 0:512], vtgt[:, 0:512], v1[:], OP.add)
                    v2 = psA.tile([1, 512], f32, tag="A")
                    nc.tensor.matmul(v2[:], es_r[:], xnat[:, 512:D], start=True, stop=True)
                    nc.vector.tensor_tensor(vtgt[:, 512:D], vtgt[:, 512:D], v2[:], OP.add)
                    if t == 11:
                        # bounce the tiles-0..11 partial through DRAM early so its
                        # round-trip hides under the remaining tiles
                        with tc.tile_pool(name="dscA", bufs=1, space="DRAM") as dscA:
                            va_dram = dscA.tile([1, D], f32, name="va_dram")
                            nc.sync.dma_start(va_dram[:], v_sb[:])
                            nc.sync.dma_start(
                                vTa_f[:], va_dram[:].rearrange("1 (c p) -> p c", p=P)
                            )

                    # inp2 tile -> out[:, 0:1024]
                    inp2 = wk.tile([P, HID], f32, tag="inp2")
                    nc.scalar.copy(inp2[:, 0:512], ia[:])
                    nc.scalar.copy(inp2[:, 512:HID], ib[:])
                    nc.sync.dma_start(out_d[rows, 0:HID], inp2[:])

                    # E^T chunks
                    et = wk3.tile([P, LM], f32r, tag="et")
                    for j in range(MC):
                        pt = psB.tile([P, P], f32r, tag="tr")
                        nc.tensor.transpose(pt[:], E[:, j * P : (j + 1) * P], ident_r[:])
                        if j % 2 == 0:
                            nc.vector.tensor_copy(et[:, j * P : (j + 1) * P], pt[:])
                        else:
                            nc.scalar.copy(et[:, j * P : (j + 1) * P], pt[:])

                    # O1 = E_norm @ mem2 -> out[:, 1024:2048] (scaled by 1/denom on copy)
                    o1a = psA.tile([P, 512], f32, tag="A")
                    o1b = psA.tile([P, 512], f32, tag="A")
                    for j in range(MC):
                        lhs = et[:, j * P : (j + 1) * P]
                        nc.tensor.matmul(
                            o1a[:], lhs, mem2[j][:, 0:512],
                            start=(j == 0), stop=(j == MC - 1),
                        )
                        nc.tensor.matmul(
                            o1b[:], lhs, mem2[j][:, 512:HID],
                            start=(j == 0), stop=(j == MC - 1),
                        )
                    nc.scalar.mul(o1k[t][:, 0:512], o1a[:], rc[:])
                    nc.scalar.mul(o1k[t][:, 512:HID], o1b[:], rc[:])
                    nc.sync.dma_start(out_d[rows, HID : 2 * HID], o1k[t][:])

                    # out3 = inp2 * O1 -> out[:, 2048:3072]
                    out3 = wk.tile([P, HID], f32, tag="out3")
                    nc.vector.tensor_tensor(
                        out3[:, 0:512], inp2[:, 0:512], o1k[t][:, 0:512], OP.mult
                    )
                    nc.vector.tensor_tensor(
                        out3[:, 512:HID], inp2[:, 512:HID], o1k[t][:, 512:HID], OP.mult
                    )
                    nc.sync.dma_start(out_d[rows, 2 * HID : 3 * HID], out3[:])

                # ============ epilogue: output_two ============
                ep_cm = tc.tile_pool(name="ep", bufs=1)
                ep = ep_cm.__enter__()
                g1 = ep.tile([P, 1], f32, tag="g1")
                nc.vector.reduce_sum(g1[:], G[:], axis=AX)
                g1r = ep.tile([P, 1], f32r, tag="g1r")
                nc.vector.tensor_copy(g1r[:], g1[:])
                ones2_f = ep.tile([P, 2], f32, tag="ones2f")
                nc.gpsimd.memset(ones2_f[:], 1.0)
                ones2_r = ep.tile([P, 2], f32r, tag="ones2r")
                nc.vector.tensor_copy(ones2_r[:], ones2_f[:])
                zps = psA.tile([1, 2], f32, tag="A")
                nc.tensor.matmul(zps[:], g1r[:], ones2_r[:], start=True, stop=True)
                rcz = ep.tile([1, 1], f32, tag="rcz")
                nc.vector.reciprocal(rcz[:], zps[0:1, 0:1])
                # v^T [128, KC]
                with tc.tile_pool(name="dsc", bufs=1, space="DRAM") as dsc:
                    v_dram = dsc.tile([1, D], f32, name="v_dram")
                    nc.sync.dma_start(v_dram[:], vb_sb[:])
                    vT_f = ep.tile([P, KC], f32, tag="vTf")
                    nc.sync.dma_start(
                        vT_f[:], v_dram[:].rearrange("1 (c p) -> p c", p=P)
                    )
                vT = ep.tile([P, KC], f32r, tag="vT")
                nc.vector.tensor_tensor(vT_f[:], vT_f[:], vTa_f[:], OP.add)
                nc.vector.tensor_copy(vT[:], vT_f[:])
                ua = psA.tile([1, 512], f32, tag="A")
                ub = psA.tile([1, 512], f32, tag="A")
                for k in range(KC):
                    nc.tensor.matmul(
                        ua[:], vT[:, k : k + 1], w2i[k][:, 0:512],
                        start=(k == 0), stop=(k == KC - 1),
                    )
                    nc.tensor.matmul(
                        ub[:], vT[:, k : k + 1], w2i[k][:, 512:HID],
                        start=(k == 0), stop=(k == KC - 1),
                    )
                o2row = ep.tile([1, HID], f32, tag="o2row")
                nc.vector.tensor_scalar_mul(o2row[:, 0:512], ua[:], rcz[:])
                nc.vector.tensor_scalar_mul(o2row[:, 512:HID], ub[:], rcz[:])
                o2row_r = ep.tile([1, HID], f32r, tag="o2rowr")
                nc.vector.tensor_tensor(o2row_r[:], o2row[:], b2i_row[:], OP.add)
                # broadcast across partitions via K=1 matmuls (PE is idle here)
                o2bc = ep.tile([P, HID], f32, tag="o2bc")
                ob1 = psA.tile([P, 512], f32, tag="A")
                nc.tensor.matmul(ob1[:], ones_r[:], o2row_r[:, 0:512], start=True, stop=True)
                nc.scalar.copy(o2bc[:, 0:512], ob1[:])
                ob2 = psA.tile([P, 512], f32, tag="A")
                nc.tensor.matmul(ob2[:], ones_r[:], o2row_r[:, 512:HID], start=True, stop=True)
                nc.scalar.copy(o2bc[:, 512:HID], ob2[:])
                # out4 = o2 * O1 -> out[:, 3072:4096]
                for t in range(NT):
                    rows = slice(t * P, (t + 1) * P)
                    out4 = (wk if t % 2 == 0 else wk3).tile(
                        [P, HID], f32, tag=("out3" if t % 2 == 0 else "xt"),
                        name=f"out4_{t}",
                    )
                    nc.vector.tensor_tensor(
                        out4[:, 0:512], o1k[t][:, 0:512], o2bc[:, 0:512], OP.mult
                    )
                    eng4 = nc.gpsimd if t % 2 == 1 else nc.vector
                    eng4.tensor_tensor(
                        out4[:, 512:HID], o1k[t][:, 512:HID], o2bc[:, 512:HID], OP.mult
                    )
                    nc.sync.dma_start(out_d[rows, 3 * HID : 4 * HID], out4[:])
                ep_cm.__exit__(None, None, None)

    nc.compile()
    return nc


def _get_nc():
    global _NC
    if _NC is None:
        _NC = _build()
    return _NC


def kernel(input, memory, mask, in1_w, mem1_w, in2_w, in2_b, mem2_w, mem2_b):
    nc = _get_nc()

    def f(a):
        return np.ascontiguousarray(np.asarray(a, dtype=np.float32))

    inp = f(input)
    memv = f(memory)
    maskv = f(mask)
    i1 = f(in1_w)
    m1 = f(mem1_w)
    wi = f(in2_w)
    bi = f(in2_b).reshape(1, HID)
    wm = f(mem2_w)
    bm = f(mem2_b).reshape(1, HID)

    in_maps = []
    for i in range(NCORES):
        in_maps.append(
            {
                "x": inp[i],
                "mem": memv[i],
                "mask": maskv[i].reshape(1, LM),
                "i1w": i1,
                "m1w": m1,
                "i2w": wi,
                "i2b": bi,
                "m2w": wm,
                "m2b": bm,
            }
        )

    kwargs = {}
    if TRACE:
        kwargs["trace"] = True
    res = run_bass_kernel_spmd(nc, in_maps, core_ids=list(range(NCORES)), **kwargs)
    global LAST_RESULT
    LAST_RESULT = res

    out = np.stack([r["out"] for r in res.results])
    mem2 = np.stack([r["mem2o"] for r in res.results])
    return out, mem2


# revision 35
# speedup vs baseline: 1.0715x; 1.0283x over previous
"""BiAttention TRN2 Bass kernel.

Data-parallel over batch N=8: one batch element per NeuronCore.

Per core (X = input [2048,1024], M = memory [512,1024]):
  cross  = (X @ M^T)                    (fp32r matmuls, contraction over D)
  att*32 = cross + 32*(mem_dot + NEG*(mask-1))   accumulated in PSUM
  E      = softmax over memory axis (exp on ScalarE with fused row-sum)
  inp2   = X @ W2i + b2i
  mem2   = M @ W2m + b2m
  O1     = E_norm @ mem2   (E transposed via PE, then fp32r matmul)
  w2     = softmax over LD of (in_dot + rowmax(att))  -> output_two via
           v = sum_l exp(s_l) X[l,:]  (PE), U = v @ W2i, o2 = U/Z + b2i
  out    = [inp2 | O1 | inp2*O1 | o2*O1]  (concat on free axis)

Transposes of X/M (fp32r) and E (fp32) go through the PE array (exact
pass-through); PSUM->SBUF copies round to float32r where matmul inputs
need it. float32r matmul measured at ~1.6e-4 max rel err on HW.
"""

import os
import sys

import numpy as np

for _p in ("/opt/trn_rl_repo", "/root/.axon_site/_ro/trn_rl_repo"):
    if os.path.isdir(_p) and _p not in sys.path:
        sys.path.insert(0, _p)

import concourse.bacc as bacc  # noqa: E402
import concourse.tile as tile  # noqa: E402
from concourse import bass_isa  # noqa: E402
from concourse import mybir  # noqa: E402
from concourse.bass_utils import run_bass_kernel_spmd  # noqa: E402
from concourse.masks import make_identity  # noqa: E402

P = 128
D = 1024
LD = 2048
LM = 512
HID = 1024
KC = D // P  # 8 contraction chunks
NT = LD // P  # 16 LD tiles
MC = LM // P  # 4 memory chunks
NCORES = 8
NEG = 1.0e30
RSCALE = 1.0 / 32.0  # 1/sqrt(D)

f32 = mybir.dt.float32
f32r = mybir.dt.float32r
AX = mybir.AxisListType.X
OP = mybir.AluOpType
EXP = mybir.ActivationFunctionType.Exp

TRACE = False
LAST_RESULT = None
_NC = None


def _build():
    nc = bacc.Bacc("TRN2", target_bir_lowering=False, debug=False)
    x_d = nc.dram_tensor("x", [LD, D], f32r, kind="ExternalInput")
    mem_d = nc.dram_tensor("mem", [LM, D], f32r, kind="ExternalInput")
    mask_d = nc.dram_tensor("mask", [1, LM], f32, kind="ExternalInput")
    i1w_d = nc.dram_tensor("i1w", [D], f32, kind="ExternalInput")
    m1w_d = nc.dram_tensor("m1w", [D], f32r, kind="ExternalInput")
    i2w_d = nc.dram_tensor("i2w", [D, HID], f32r, kind="ExternalInput")
    i2b_d = nc.dram_tensor("i2b", [1, HID], f32, kind="ExternalInput")
    m2w_d = nc.dram_tensor("m2w", [D, HID], f32r, kind="ExternalInput")
    m2b_d = nc.dram_tensor("m2b", [1, HID], f32, kind="ExternalInput")
    out_d = nc.dram_tensor("out", [LD, 4 * HID], f32, kind="ExternalOutput")
    mem2_d = nc.dram_tensor("mem2o", [LM, HID], f32r, kind="ExternalOutput")

    with tile.TileContext(nc) as tc:
        with (
            tc.tile_pool(name="persist", bufs=1) as pp,
            tc.tile_pool(name="psA", bufs=6, space="PSUM") as psA,
            tc.tile_pool(name="psB", bufs=2, space="PSUM") as psB,
        ):
            # ---- constants ----
            ident = pp.tile([P, P], f32, tag="ident")
            make_identity(nc, ident[:])
            ident_r = pp.tile([P, P], f32r, tag="identr")
            nc.vector.tensor_copy(ident_r[:], ident[:])
            ones_f = pp.tile([1, P], f32, tag="onesf")
            nc.gpsimd.memset(ones_f[:], 1.0)
            ones_r = pp.tile([1, P], f32r, tag="onesr")
            nc.vector.tensor_copy(ones_r[:], ones_f[:])

            # ---- persistent tensors ----
            mt = [
                pp.tile([P, LM], f32r, tag=f"mt{k}", name=f"mt{k}") for k in range(KC)
            ]
            mem2 = [
                pp.tile([P, HID], f32r, tag=f"mem2_{j}", name=f"mem2_{j}")
                for j in range(MC)
            ]
            o1k = [
                pp.tile([P, HID], f32, tag=f"o1k{t}", name=f"o1k{t}") for t in range(NT)
            ]
            G = pp.tile([P, NT], f32, tag="G")
            v_sb = pp.tile([1, D], f32, tag="vsb")
            nc.gpsimd.memset(v_sb[:], 0.0)
            vTa_f = pp.tile([P, KC], f32, tag="vTaf")
            row_bias = pp.tile([1, LM], f32r, tag="rowbias")

            # ================= preamble =================
            with tc.tile_pool(name="pre", bufs=1) as pre:
                # transpose memory -> mt[k] = M^T chunk [128(d), 512(m)]
                for j in range(MC):
                    memt = pre.tile([P, D], f32r, tag=f"memt{j % 2}", name=f"memt{j}")
                    nc.sync.dma_start(memt[:], mem_d[j * P : (j + 1) * P, :])
                    for k in range(KC):
                        pt = psB.tile([P, P], f32r, tag="tr")
                        nc.tensor.transpose(pt[:], memt[:, k * P : (k + 1) * P], ident_r[:])
                        if k % 2 == 0:
                            nc.vector.tensor_copy(mt[k][:, j * P : (j + 1) * P], pt[:])
                        else:
                            nc.scalar.copy(mt[k][:, j * P : (j + 1) * P], pt[:])
                # ---- small weights (issued after the memory loads) ----
                m1w = pp.tile([P, KC], f32r, tag="m1w")
                nc.sync.dma_start(m1w[:], m1w_d[:].rearrange("(c p) -> p c", p=P))
                mask_row = pp.tile([1, LM], f32, tag="maskrow")
                nc.sync.dma_start(mask_row[:], mask_d[:])
                b2i_row = pp.tile([1, HID], f32, tag="b2irow")
                nc.sync.dma_start(b2i_row[:], i2b_d[:])
                b2i_row_r = pp.tile([1, HID], f32r, tag="b2irowr")
                nc.vector.tensor_copy(b2i_row_r[:], b2i_row[:])
                i1w_f = pp.tile([P, 2 * KC], f32, tag="i1wf")
                nc.gpsimd.memset(i1w_f[:], 0.0)
                nc.sync.dma_start(
                    i1w_f[:, 0 : 2 * KC : 2], i1w_d[:].rearrange("(c p) -> p c", p=P)
                )
                i1w = pp.tile([P, 2 * KC], f32r, tag="i1w")
                nc.vector.tensor_copy(i1w[:], i1w_f[:])

                # mem_dot row [1, LM]
                mdp = psA.tile([1, LM], f32, tag="A")
                for k in range(KC):
                    nc.tensor.matmul(
                        mdp[:], m1w[:, k : k + 1], mt[k][:],
                        start=(k == 0), stop=(k == KC - 1),
                    )
                # row_bias = 32*mem_dot + 32*NEG*(mask-1)
                maskt = pre.tile([1, LM], f32, tag="maskt")
                nc.vector.tensor_scalar(
                    maskt[:], mask_row[:], 32.0 * NEG, -32.0 * NEG, OP.mult, OP.add
                )
                md32 = pre.tile([1, LM], f32, tag="md32")
                nc.vector.tensor_scalar_mul(md32[:], mdp[:], 32.0)
                nc.vector.tensor_tensor(row_bias[:], md32[:], maskt[:], OP.add)


                # mem2 = M @ W2m + b2m
                b2m_row = pre.tile([1, HID], f32, tag="b2mrow")
                nc.sync.dma_start(b2m_row[:], m2b_d[:])
                b2m_row_r = pre.tile([1, HID], f32r, tag="b2mrowr")
                nc.vector.tensor_copy(b2m_row_r[:], b2m_row[:])
                # interleave W2i (tile-0 k-loop) and W2m (mem2) chunk loads so
                # both consumers start as their first chunks land
                w2i = []
                w2m = []
                for k in range(KC):
                    wm = pre.tile([P, HID], f32r, tag=f"w2m{k}", name=f"w2m{k}")
                    nc.sync.dma_start(wm[:], m2w_d[k * P : (k + 1) * P, :])
                    w2m.append(wm)
                    wi = pp.tile([P, HID], f32r, tag=f"w2i{k}", name=f"w2i{k}")
                    nc.sync.dma_start(wi[:], i2w_d[k * P : (k + 1) * P, :])
                    w2i.append(wi)
                for j in range(MC):
                    pa = psA.tile([P, 512], f32, tag="A")
                    pb = psA.tile([P, 512], f32, tag="A")
                    nc.tensor.matmul(pa[:], ones_r[:], b2m_row_r[:, 0:512], start=True, stop=False)
                    nc.tensor.matmul(pb[:], ones_r[:], b2m_row_r[:, 512:HID], start=True, stop=False)
                    for k in range(KC):
                        lhs = mt[k][:, j * P : (j + 1) * P]
                        nc.tensor.matmul(
                            pa[:], lhs, w2m[k][:, 0:512],
                            start=False, stop=(k == KC - 1),
                        )
                        nc.tensor.matmul(
                            pb[:], lhs, w2m[k][:, 512:HID],
                            start=False, stop=(k == KC - 1),
                        )
                    nc.scalar.copy(mem2[j][:, 0:512], pa[:])
                    nc.scalar.copy(mem2[j][:, 512:HID], pb[:])
                    nc.gpsimd.dma_start(mem2_d[j * P : (j + 1) * P, :], mem2[j][:])

            # ================= main loop over LD tiles =================
            with tc.tile_pool(name="wk", bufs=2) as wk, tc.tile_pool(name="wk3", bufs=3) as wk3:
                for t in range(NT):
                    rows = slice(t * P, (t + 1) * P)
                    xnat = wk.tile([P, D], f32r, tag="xnat")
                    nc.gpsimd.dma_start(xnat[:], x_d[rows, :])
                    # X^T chunks: xt[:, k*P:(k+1)*P] = [128(d of chunk k), 128(l)]
                    xt = wk3.tile([P, D], f32r, tag="xt")
                    for k in range(KC):
                        pt = psB.tile([P, P], f32r, tag="tr")
                        nc.tensor.transpose(pt[:], xnat[:, k * P : (k + 1) * P], ident_r[:])
                        if k % 2 == 0:
                            nc.vector.tensor_copy(xt[:, k * P : (k + 1) * P], pt[:])
                        else:
                            nc.scalar.copy(xt[:, k * P : (k + 1) * P], pt[:])

                    cr = psA.tile([P, LM], f32, tag="A")
                    ia = psA.tile([P, 512], f32, tag="A")
                    ib = psA.tile([P, 512], f32, tag="A")
                    idp = psA.tile([P, 2], f32, tag="A")
                    # bias row first, then cross chunks; the group closes at the
                    # last cross matmul so softmax starts as early as possible
                    nc.tensor.matmul(cr[:], ones_r[:], row_bias[:], start=True, stop=False)
                    nc.tensor.matmul(ia[:], ones_r[:], b2i_row_r[:, 0:512], start=True, stop=False)
                    nc.tensor.matmul(ib[:], ones_r[:], b2i_row_r[:, 512:HID], start=True, stop=False)
                    for k in range(KC):
                        nc.tensor.matmul(
                            cr[:], xt[:, k * P : (k + 1) * P], mt[k][:],
                            start=False, stop=(k == KC - 1),
                        )
                    for k in range(KC):
                        lhs = xt[:, k * P : (k + 1) * P]
                        nc.tensor.matmul(
                            ia[:], lhs, w2i[k][:, 0:512],
                            start=False, stop=(k == KC - 1),
                        )
                        nc.tensor.matmul(
                            ib[:], lhs, w2i[k][:, 512:HID],
                            start=False, stop=(k == KC - 1),
                        )
                        nc.tensor.matmul(
                            idp[:], lhs, i1w[:, 2 * k : 2 * k + 2],
                            start=(k == 0), stop=(k == KC - 1),
                        )

                    # softmax over the memory axis. att values are O(10), so
                    # exp() cannot overflow: skip the max-subtraction for E and
                    # keep rowmax only for the weight_two statistic (it is part
                    # of the math there, not just stabilization). This takes the
                    # DVE reduce off the cr->E critical path.
                    E = wk3.tile([P, LM], f32r, tag="E")
                    dsum = wk.tile([P, 1], f32, tag="dsum")
                    nc.scalar.activation(
                        E[:], cr[:], EXP, scale=RSCALE, accum_out=dsum[:]
                    )
                    nmx = wk.tile([P, 1], f32, tag="nmx")
                    nc.vector.reduce_max(nmx[:], cr[:], axis=AX, negate=True)
                    nbias = wk.tile([P, 1], f32, tag="nbias")
                    nc.vector.tensor_scalar_mul(nbias[:], nmx[:], RSCALE)
                    rc = wk.tile([P, 1], f32, tag="rc")
                    nc.vector.reciprocal(rc[:], dsum[:])
                    # s = in_dot + rowmax/32 ; exp(s) -> G column t
                    s_t = wk.tile([P, 1], f32, tag="s")
                    nc.vector.tensor_tensor(s_t[:], idp[:, 0:1], nbias[:], OP.subtract)
                    nc.scalar.activation(G[:, t : t + 1], s_t[:], EXP)
                    es_r = wk.tile([P, 1], f32r, tag="esr")
                    nc.vector.tensor_copy(es_r[:], G[:, t : t + 1])

                    # v += exp(s)^T @ X   (for output_two)
                    if t < NT - 1:
                        v1 = psA.tile([1, 512], f32, tag="A")
                        nc.tensor.matmul(v1[:], es_r[:], xnat[:, 0:512], start=True, stop=True)
                        nc.vector.tensor_tensor(v_sb[:, 0:512], v_sb[:, 0:512], v1[:], OP.add)
                        v2 = psA.tile([1, 512], f32, tag="A")
                        nc.tensor.matmul(v2[:], es_r[:], xnat[:, 512:D], start=True, stop=True)
                        nc.vector.tensor_tensor(v_sb[:, 512:D], v_sb[:, 512:D], v2[:], OP.add)
                    else:
                        # last tile: accumulate directly in transposed form so the
                        # epilogue needs no DRAM round-trip for this part
                        es2 = wk.tile([P, 2], f32r, tag="es2")
                        nc.vector.tensor_copy(es2[:, 0:1], G[:, t : t + 1])
                        nc.vector.tensor_copy(es2[:, 1:2], G[:, t : t + 1])
                        vtb = psA.tile([P, 2 * KC], f32, tag="A")
                        for k in range(KC):
                            nc.tensor.matmul(
                                vtb[:, 2 * k : 2 * k + 2],
                                xnat[:, k * P : (k + 1) * P], es2[:],
                                start=True, stop=True,
                            )
                        vtb_f = wk.tile([P, KC], f32, tag="vtbf")
                        nc.vector.tensor_copy(vtb_f[:], vtb[:, 0 : 2 * KC : 2])
                    if t == NT - 2:
                        # bounce the tiles-0..14 partial through DRAM early so its
                        # round-trip hides under the last tile
                        with tc.tile_pool(name="dscA", bufs=1, space="DRAM") as dscA:
                            va_dram = dscA.tile([1, D], f32, name="va_dram")
                            nc.sync.dma_start(va_dram[:], v_sb[:])
                            nc.sync.dma_start(
                                vTa_f[:], va_dram[:].rearrange("1 (c p) -> p c", p=P)
                            )

                    # inp2 tile -> out[:, 0:1024]
                    inp2 = wk.tile([P, HID], f32, tag="inp2")
                    nc.scalar.copy(inp2[:, 0:512], ia[:])
                    nc.scalar.copy(inp2[:, 512:HID], ib[:])
                    nc.sync.dma_start(out_d[rows, 0:HID], inp2[:])

                    # E^T chunks
                    et = wk3.tile([P, LM], f32r, tag="et")
                    for j in range(MC):
                        pt = psB.tile([P, P], f32r, tag="tr")
                        nc.tensor.transpose(pt[:], E[:, j * P : (j + 1) * P], ident_r[:])
                        if j % 2 == 0:
                            nc.vector.tensor_copy(et[:, j * P : (j + 1) * P], pt[:])
                        else:
                            nc.scalar.copy(et[:, j * P : (j + 1) * P], pt[:])

                    # O1 = E_norm @ mem2 -> out[:, 1024:2048] (scaled by 1/denom on copy)
                    o1a = psA.tile([P, 512], f32, tag="A")
                    o1b = psA.tile([P, 512], f32, tag="A")
                    for j in range(MC):
                        lhs = et[:, j * P : (j + 1) * P]
                        nc.tensor.matmul(
                            o1a[:], lhs, mem2[j][:, 0:512],
                            start=(j == 0), stop=(j == MC - 1),
                        )
                        nc.tensor.matmul(
                            o1b[:], lhs, mem2[j][:, 512:HID],
                            start=(j == 0), stop=(j == MC - 1),
                        )
                    nc.scalar.mul(o1k[t][:, 0:512], o1a[:], rc[:])
                    nc.scalar.mul(o1k[t][:, 512:HID], o1b[:], rc[:])
                    nc.sync.dma_start(out_d[rows, HID : 2 * HID], o1k[t][:])

                    # out3 = inp2 * O1 -> out[:, 2048:3072]
                    out3 = wk.tile([P, HID], f32, tag="out3")
                    nc.vector.tensor_tensor(
                        out3[:, 0:512], inp2[:, 0:512], o1k[t][:, 0:512], OP.mult
                    )
                    nc.vector.tensor_tensor(
                        out3[:, 512:HID], inp2[:, 512:HID], o1k[t][:, 512:HID], OP.mult
                    )
                    nc.sync.dma_start(out_d[rows, 2 * HID : 3 * HID], out3[:])

                # ============ epilogue: output_two ============
                ep_cm = tc.tile_pool(name="ep", bufs=1)
                ep = ep_cm.__enter__()
                g1 = ep.tile([P, 1], f32, tag="g1")
                nc.vector.reduce_sum(g1[:], G[:], axis=AX)
                g1r = ep.tile([P, 1], f32r, tag="g1r")
                nc.vector.tensor_copy(g1r[:], g1[:])
                ones2_f = ep.tile([P, 2], f32, tag="ones2f")
                nc.gpsimd.memset(ones2_f[:], 1.0)
                ones2_r = ep.tile([P, 2], f32r, tag="ones2r")
                nc.vector.tensor_copy(ones2_r[:], ones2_f[:])
                zps = psA.tile([1, 2], f32, tag="A")
                nc.tensor.matmul(zps[:], g1r[:], ones2_r[:], start=True, stop=True)
                rcz = ep.tile([1, 1], f32, tag="rcz")
                nc.vector.reciprocal(rcz[:], zps[0:1, 0:1])
                # v^T [128, KC]
                vT = ep.tile([P, KC], f32r, tag="vT")
                nc.vector.tensor_tensor(vtb_f[:], vtb_f[:], vTa_f[:], OP.add)
                nc.vector.tensor_copy(vT[:], vtb_f[:])
                ua = psA.tile([1, 512], f32, tag="A")
                ub = psA.tile([1, 512], f32, tag="A")
                for k in range(KC):
                    nc.tensor.matmul(
                        ua[:], vT[:, k : k + 1], w2i[k][:, 0:512],
                        start=(k == 0), stop=(k == KC - 1),
                    )
                    nc.tensor.matmul(
                        ub[:], vT[:, k : k + 1], w2i[k][:, 512:HID],
                        start=(k == 0), stop=(k == KC - 1),
                    )
                o2row = ep.tile([1, HID], f32, tag="o2row")
                nc.vector.tensor_scalar_mul(o2row[:, 0:512], ua[:], rcz[:])
                nc.vector.tensor_scalar_mul(o2row[:, 512:HID], ub[:], rcz[:])
                o2row_r = ep.tile([1, HID], f32r, tag="o2rowr")
                nc.vector.tensor_tensor(o2row_r[:], o2row[:], b2i_row[:], OP.add)
                # broadcast across partitions via K=1 matmuls (PE is idle here)
                o2bc = ep.tile([P, HID], f32, tag="o2bc")
                ob1 = psA.tile([P, 512], f32, tag="A")
                nc.tensor.matmul(ob1[:], ones_r[:], o2row_r[:, 0:512], start=True, stop=True)
                nc.scalar.copy(o2bc[:, 0:512], ob1[:])
                ob2 = psA.tile([P, 512], f32, tag="A")
                nc.tensor.matmul(ob2[:], ones_r[:], o2row_r[:, 512:HID], start=True, stop=True)
                nc.scalar.copy(o2bc[:, 512:HID], ob2[:])
                # out4 = o2 * O1 -> out[:, 3072:4096]
                for t in range(NT):
                    rows = slice(t * P, (t + 1) * P)
                    out4 = (wk if t % 2 == 0 else wk3).tile(
                        [P, HID], f32, tag=("out3" if t % 2 == 0 else "xt"),
                        name=f"out4_{t}",
                    )
                    nc.vector.tensor_tensor(
                        out4[:, 0:512], o1k[t][:, 0:512], o2bc[:, 0:512], OP.mult
                    )
                    eng4 = nc.gpsimd if t % 2 == 1 else nc.vector
                    eng4.tensor_tensor(
                        out4[:, 512:HID], o1k[t][:, 512:HID], o2bc[:, 512:HID], OP.mult
                    )
                    nc.sync.dma_start(out_d[rows, 3 * HID : 4 * HID], out4[:])
                ep_cm.__exit__(None, None, None)

    nc.compile()
    return nc


def _get_nc():
    global _NC
    if _NC is None:
        _NC = _build()
    return _NC


def kernel(input, memory, mask, in1_w, mem1_w, in2_w, in2_b, mem2_w, mem2_b):
    nc = _get_nc()

    def f(a):
        return np.ascontiguousarray(np.asarray(a, dtype=np.float32))

    inp = f(input)
    memv = f(memory)
    maskv = f(mask)
    i1 = f(in1_w)
    m1 = f(mem1_w)
    wi = f(in2_w)
    bi = f(in2_b).reshape(1, HID)
    wm = f(mem2_w)
    bm = f(mem2_b).reshape(1, HID)

    in_maps = []
    for i in range(NCORES):
        in_maps.append(
            {
                "x": inp[i],
                "mem": memv[i],
                "mask": maskv[i].reshape(1, LM),
                "i1w": i1,
                "m1w": m1,
                "i2w": wi,
                "i2b": bi,
                "m2w": wm,
                "m2b": bm,
            }
        )

    kwargs = {}
    if TRACE:
        kwargs["trace"] = True
    res = run_bass_kernel_spmd(nc, in_maps, core_ids=list(range(NCORES)), **kwargs)
    global LAST_RESULT
    LAST_RESULT = res

    out = np.stack([r["out"] for r in res.results])
    mem2 = np.stack([r["mem2o"] for r in res.results])
    return out, mem2


# revision 37
# speedup vs baseline: 1.0924x; 1.0195x over previous
"""BiAttention TRN2 Bass kernel.

Data-parallel over batch N=8: one batch element per NeuronCore.

Per core (X = input [2048,1024], M = memory [512,1024]):
  cross  = (X @ M^T)                    (fp32r matmuls, contraction over D)
  att*32 = cross + 32*(mem_dot + NEG*(mask-1))   accumulated in PSUM
  E      = softmax over memory axis (exp on ScalarE with fused row-sum)
  inp2   = X @ W2i + b2i
  mem2   = M @ W2m + b2m
  O1     = E_norm @ mem2   (E transposed via PE, then fp32r matmul)
  w2     = softmax over LD of (in_dot + rowmax(att))  -> output_two via
           v = sum_l exp(s_l) X[l,:]  (PE), U = v @ W2i, o2 = U/Z + b2i
  out    = [inp2 | O1 | inp2*O1 | o2*O1]  (concat on free axis)

Transposes of X/M (fp32r) and E (fp32) go through the PE array (exact
pass-through); PSUM->SBUF copies round to float32r where matmul inputs
need it. float32r matmul measured at ~1.6e-4 max rel err on HW.
"""

import os
import sys

import numpy as np

for _p in ("/opt/trn_rl_repo", "/root/.axon_site/_ro/trn_rl_repo"):
    if os.path.isdir(_p) and _p not in sys.path:
        sys.path.insert(0, _p)

import concourse.bacc as bacc  # noqa: E402
import concourse.tile as tile  # noqa: E402
from concourse import bass_isa  # noqa: E402
from concourse import mybir  # noqa: E402
from concourse.bass_utils import run_bass_kernel_spmd  # noqa: E402
from concourse.masks import make_identity  # noqa: E402

P = 128
D = 1024
LD = 2048
LM = 512
HID = 1024
KC = D // P  # 8 contraction chunks
NT = LD // P  # 16 LD tiles
MC = LM // P  # 4 memory chunks
NCORES = 8
NEG = 1.0e30
RSCALE = 1.0 / 32.0  # 1/sqrt(D)

f32 = mybir.dt.float32
f32r = mybir.dt.float32r
AX = mybir.AxisListType.X
OP = mybir.AluOpType
EXP = mybir.ActivationFunctionType.Exp

TRACE = False
LAST_RESULT = None
_NC = None


def _build():
    nc = bacc.Bacc("TRN2", target_bir_lowering=False, debug=False)
    x_d = nc.dram_tensor("x", [LD, D], f32r, kind="ExternalInput")
    mem_d = nc.dram_tensor("mem", [LM, D], f32r, kind="ExternalInput")
    mask_d = nc.dram_tensor("mask", [1, LM], f32, kind="ExternalInput")
    i1w_d = nc.dram_tensor("i1w", [D], f32, kind="ExternalInput")
    m1w_d = nc.dram_tensor("m1w", [D], f32r, kind="ExternalInput")
    i2w_d = nc.dram_tensor("i2w", [D, HID], f32r, kind="ExternalInput")
    i2b_d = nc.dram_tensor("i2b", [1, HID], f32, kind="ExternalInput")
    m2w_d = nc.dram_tensor("m2w", [D, HID], f32r, kind="ExternalInput")
    m2b_d = nc.dram_tensor("m2b", [1, HID], f32, kind="ExternalInput")
    out_d = nc.dram_tensor("out", [LD, 4 * HID], f32, kind="ExternalOutput")
    mem2_d = nc.dram_tensor("mem2o", [LM, HID], f32r, kind="ExternalOutput")

    with tile.TileContext(nc) as tc:
        with (
            tc.tile_pool(name="persist", bufs=1) as pp,
            tc.tile_pool(name="psA", bufs=6, space="PSUM") as psA,
            tc.tile_pool(name="psB", bufs=2, space="PSUM") as psB,
        ):
            # ---- constants ----
            ident = pp.tile([P, P], f32, tag="ident")
            make_identity(nc, ident[:])
            ident_r = pp.tile([P, P], f32r, tag="identr")
            nc.vector.tensor_copy(ident_r[:], ident[:])
            ones_f = pp.tile([1, P], f32, tag="onesf")
            nc.gpsimd.memset(ones_f[:], 1.0)
            ones_r = pp.tile([1, P], f32r, tag="onesr")
            nc.vector.tensor_copy(ones_r[:], ones_f[:])

            # ---- persistent tensors ----
            mt = [
                pp.tile([P, LM], f32r, tag=f"mt{k}", name=f"mt{k}") for k in range(KC)
            ]
            mem2 = [
                pp.tile([P, HID], f32r, tag=f"mem2_{j}", name=f"mem2_{j}")
                for j in range(MC)
            ]
            o1k = [
                pp.tile([P, HID], f32, tag=f"o1k{t}", name=f"o1k{t}") for t in range(NT)
            ]
            G = pp.tile([P, NT], f32, tag="G")
            v_sb = pp.tile([1, D], f32, tag="vsb")
            nc.gpsimd.memset(v_sb[:], 0.0)
            vTa_f = pp.tile([P, KC], f32, tag="vTaf")
            row_bias = pp.tile([1, LM], f32r, tag="rowbias")

            # ================= preamble =================
            with tc.tile_pool(name="pre", bufs=1) as pre:
                # transpose memory -> mt[k] = M^T chunk [128(d), 512(m)]
                for j in range(MC):
                    memt = pre.tile([P, D], f32r, tag=f"memt{j % 2}", name=f"memt{j}")
                    nc.sync.dma_start(memt[:], mem_d[j * P : (j + 1) * P, :])
                    for k in range(KC):
                        pt = psB.tile([P, P], f32r, tag="tr")
                        nc.tensor.transpose(pt[:], memt[:, k * P : (k + 1) * P], ident_r[:])
                        if k % 2 == 0:
                            nc.vector.tensor_copy(mt[k][:, j * P : (j + 1) * P], pt[:])
                        else:
                            nc.scalar.copy(mt[k][:, j * P : (j + 1) * P], pt[:])
                # ---- small weights (issued after the memory loads) ----
                m1w = pp.tile([P, KC], f32r, tag="m1w")
                nc.sync.dma_start(m1w[:], m1w_d[:].rearrange("(c p) -> p c", p=P))
                mask_row = pp.tile([1, LM], f32, tag="maskrow")
                nc.sync.dma_start(mask_row[:], mask_d[:])
                b2i_row = pp.tile([1, HID], f32, tag="b2irow")
                nc.sync.dma_start(b2i_row[:], i2b_d[:])
                b2i_row_r = pp.tile([1, HID], f32r, tag="b2irowr")
                nc.vector.tensor_copy(b2i_row_r[:], b2i_row[:])
                i1w_f = pp.tile([P, 2 * KC], f32, tag="i1wf")
                nc.gpsimd.memset(i1w_f[:], 0.0)
                nc.sync.dma_start(
                    i1w_f[:, 0 : 2 * KC : 2], i1w_d[:].rearrange("(c p) -> p c", p=P)
                )
                i1w = pp.tile([P, 2 * KC], f32r, tag="i1w")
                nc.vector.tensor_copy(i1w[:], i1w_f[:])

                # mem_dot row [1, LM]
                mdp = psA.tile([1, LM], f32, tag="A")
                for k in range(KC):
                    nc.tensor.matmul(
                        mdp[:], m1w[:, k : k + 1], mt[k][:],
                        start=(k == 0), stop=(k == KC - 1),
                    )
                # row_bias = 32*mem_dot + 32*NEG*(mask-1)
                maskt = pre.tile([1, LM], f32, tag="maskt")
                nc.vector.tensor_scalar(
                    maskt[:], mask_row[:], 32.0 * NEG, -32.0 * NEG, OP.mult, OP.add
                )
                md32 = pre.tile([1, LM], f32, tag="md32")
                nc.vector.tensor_scalar_mul(md32[:], mdp[:], 32.0)
                nc.vector.tensor_tensor(row_bias[:], md32[:], maskt[:], OP.add)


                # mem2 = M @ W2m + b2m
                b2m_row = pre.tile([1, HID], f32, tag="b2mrow")
                nc.sync.dma_start(b2m_row[:], m2b_d[:])
                b2m_row_r = pre.tile([1, HID], f32r, tag="b2mrowr")
                nc.vector.tensor_copy(b2m_row_r[:], b2m_row[:])
                # interleave W2i (tile-0 k-loop) and W2m (mem2) chunk loads so
                # both consumers start as their first chunks land
                w2i = []
                w2m = []
                for k in range(KC):
                    wm = pre.tile([P, HID], f32r, tag=f"w2m{k}", name=f"w2m{k}")
                    nc.sync.dma_start(wm[:], m2w_d[k * P : (k + 1) * P, :])
                    w2m.append(wm)
                    wi = pp.tile([P, HID], f32r, tag=f"w2i{k}", name=f"w2i{k}")
                    nc.sync.dma_start(wi[:], i2w_d[k * P : (k + 1) * P, :])
                    w2i.append(wi)
                for j in range(MC):
                    pa = psA.tile([P, 512], f32, tag="A")
                    pb = psA.tile([P, 512], f32, tag="A")
                    nc.tensor.matmul(pa[:], ones_r[:], b2m_row_r[:, 0:512], start=True, stop=False)
                    nc.tensor.matmul(pb[:], ones_r[:], b2m_row_r[:, 512:HID], start=True, stop=False)
                    for k in range(KC):
                        lhs = mt[k][:, j * P : (j + 1) * P]
                        nc.tensor.matmul(
                            pa[:], lhs, w2m[k][:, 0:512],
                            start=False, stop=(k == KC - 1),
                        )
                        nc.tensor.matmul(
                            pb[:], lhs, w2m[k][:, 512:HID],
                            start=False, stop=(k == KC - 1),
                        )
                    nc.scalar.copy(mem2[j][:, 0:512], pa[:])
                    nc.scalar.copy(mem2[j][:, 512:HID], pb[:])
                    nc.gpsimd.dma_start(mem2_d[j * P : (j + 1) * P, :], mem2[j][:])

            # ================= main loop over LD tiles =================
            with tc.tile_pool(name="wk", bufs=2) as wk, tc.tile_pool(name="wk3", bufs=3) as wk3:
                for t in range(NT):
                    rows = slice(t * P, (t + 1) * P)
                    xnat = wk3.tile([P, D], f32r, tag="xnat")
                    nc.gpsimd.dma_start(xnat[:], x_d[rows, :])
                    # X^T chunks: xt[:, k*P:(k+1)*P] = [128(d of chunk k), 128(l)]
                    xt = wk3.tile([P, D], f32r, tag="xt")
                    for k in range(KC):
                        pt = psB.tile([P, P], f32r, tag="tr")
                        nc.tensor.transpose(pt[:], xnat[:, k * P : (k + 1) * P], ident_r[:])
                        if k % 2 == 0:
                            nc.vector.tensor_copy(xt[:, k * P : (k + 1) * P], pt[:])
                        else:
                            nc.scalar.copy(xt[:, k * P : (k + 1) * P], pt[:])

                    cr = psA.tile([P, LM], f32, tag="A")
                    ia = psA.tile([P, 512], f32, tag="A")
                    ib = psA.tile([P, 512], f32, tag="A")
                    idp = psA.tile([P, 2], f32, tag="A")
                    # bias row first, then cross chunks; the group closes at the
                    # last cross matmul so softmax starts as early as possible
                    nc.tensor.matmul(cr[:], ones_r[:], row_bias[:], start=True, stop=False)
                    nc.tensor.matmul(ia[:], ones_r[:], b2i_row_r[:, 0:512], start=True, stop=False)
                    nc.tensor.matmul(ib[:], ones_r[:], b2i_row_r[:, 512:HID], start=True, stop=False)
                    for k in range(KC):
                        nc.tensor.matmul(
                            cr[:], xt[:, k * P : (k + 1) * P], mt[k][:],
                            start=False, stop=(k == KC - 1),
                        )
                    for k in range(KC):
                        lhs = xt[:, k * P : (k + 1) * P]
                        nc.tensor.matmul(
                            ia[:], lhs, w2i[k][:, 0:512],
                            start=False, stop=(k == KC - 1),
                        )
                        nc.tensor.matmul(
                            ib[:], lhs, w2i[k][:, 512:HID],
                            start=False, stop=(k == KC - 1),
                        )
                        nc.tensor.matmul(
                            idp[:], lhs, i1w[:, 2 * k : 2 * k + 2],
                            start=(k == 0), stop=(k == KC - 1),
                        )

                    # softmax over the memory axis. att values are O(10), so
                    # exp() cannot overflow: skip the max-subtraction for E and
                    # keep rowmax only for the weight_two statistic (it is part
                    # of the math there, not just stabilization). This takes the
                    # DVE reduce off the cr->E critical path.
                    E = wk3.tile([P, LM], f32r, tag="E")
                    dsum = wk.tile([P, 1], f32, tag="dsum")
                    nc.scalar.activation(
                        E[:], cr[:], EXP, scale=RSCALE, accum_out=dsum[:]
                    )
                    nmx = wk.tile([P, 1], f32, tag="nmx")
                    nc.vector.reduce_max(nmx[:], cr[:], axis=AX, negate=True)
                    nbias = wk.tile([P, 1], f32, tag="nbias")
                    nc.vector.tensor_scalar_mul(nbias[:], nmx[:], RSCALE)
                    rc = wk.tile([P, 1], f32, tag="rc")
                    nc.vector.reciprocal(rc[:], dsum[:])
                    # s = in_dot + rowmax/32 ; exp(s) -> G column t
                    s_t = wk.tile([P, 1], f32, tag="s")
                    nc.vector.tensor_tensor(s_t[:], idp[:, 0:1], nbias[:], OP.subtract)
                    nc.scalar.activation(G[:, t : t + 1], s_t[:], EXP)
                    es_r = wk.tile([P, 1], f32r, tag="esr")
                    nc.vector.tensor_copy(es_r[:], G[:, t : t + 1])

                    # v += exp(s)^T @ X   (for output_two)
                    if t < NT - 1:
                        v1 = psA.tile([1, 512], f32, tag="A")
                        nc.tensor.matmul(v1[:], es_r[:], xnat[:, 0:512], start=True, stop=True)
                        nc.vector.tensor_tensor(v_sb[:, 0:512], v_sb[:, 0:512], v1[:], OP.add)
                        v2 = psA.tile([1, 512], f32, tag="A")
                        nc.tensor.matmul(v2[:], es_r[:], xnat[:, 512:D], start=True, stop=True)
                        nc.vector.tensor_tensor(v_sb[:, 512:D], v_sb[:, 512:D], v2[:], OP.add)
                    else:
                        # last tile: accumulate directly in transposed form so the
                        # epilogue needs no DRAM round-trip for this part
                        es2 = wk.tile([P, 2], f32r, tag="es2")
                        nc.vector.tensor_copy(es2[:, 0:1], G[:, t : t + 1])
                        nc.vector.tensor_copy(es2[:, 1:2], G[:, t : t + 1])
                        vtb = psA.tile([P, 2 * KC], f32, tag="A")
                        for k in range(KC):
                            nc.tensor.matmul(
                                vtb[:, 2 * k : 2 * k + 2],
                                xnat[:, k * P : (k + 1) * P], es2[:],
                                start=True, stop=True,
                            )
                        vtb_f = wk.tile([P, KC], f32, tag="vtbf")
                        nc.vector.tensor_copy(vtb_f[:], vtb[:, 0 : 2 * KC : 2])
                    if t == NT - 2:
                        # bounce the tiles-0..14 partial through DRAM early so its
                        # round-trip hides under the last tile
                        with tc.tile_pool(name="dscA", bufs=1, space="DRAM") as dscA:
                            va_dram = dscA.tile([1, D], f32, name="va_dram")
                            nc.sync.dma_start(va_dram[:], v_sb[:])
                            nc.sync.dma_start(
                                vTa_f[:], va_dram[:].rearrange("1 (c p) -> p c", p=P)
                            )

                    # inp2 tile -> out[:, 0:1024]
                    inp2 = wk.tile([P, HID], f32, tag="inp2")
                    nc.scalar.copy(inp2[:, 0:512], ia[:])
                    nc.scalar.copy(inp2[:, 512:HID], ib[:])
                    nc.sync.dma_start(out_d[rows, 0:HID], inp2[:])

                    # E^T chunks
                    et = wk3.tile([P, LM], f32r, tag="et")
                    for j in range(MC):
                        pt = psB.tile([P, P], f32r, tag="tr")
                        nc.tensor.transpose(pt[:], E[:, j * P : (j + 1) * P], ident_r[:])
                        if j % 2 == 0:
                            nc.vector.tensor_copy(et[:, j * P : (j + 1) * P], pt[:])
                        else:
                            nc.scalar.copy(et[:, j * P : (j + 1) * P], pt[:])

                    # O1 = E_norm @ mem2 -> out[:, 1024:2048] (scaled by 1/denom on copy)
                    o1a = psA.tile([P, 512], f32, tag="A")
                    o1b = psA.tile([P, 512], f32, tag="A")
                    for j in range(MC):
                        lhs = et[:, j * P : (j + 1) * P]
                        nc.tensor.matmul(
                            o1a[:], lhs, mem2[j][:, 0:512],
                            start=(j == 0), stop=(j == MC - 1),
                        )
                        nc.tensor.matmul(
                            o1b[:], lhs, mem2[j][:, 512:HID],
                            start=(j == 0), stop=(j == MC - 1),
                        )
                    nc.scalar.mul(o1k[t][:, 0:512], o1a[:], rc[:])
                    nc.scalar.mul(o1k[t][:, 512:HID], o1b[:], rc[:])
                    nc.sync.dma_start(out_d[rows, HID : 2 * HID], o1k[t][:])

                    # out3 = inp2 * O1 -> out[:, 2048:3072]
                    out3 = wk.tile([P, HID], f32, tag="out3")
                    nc.vector.tensor_tensor(
                        out3[:, 0:512], inp2[:, 0:512], o1k[t][:, 0:512], OP.mult
                    )
                    nc.vector.tensor_tensor(
                        out3[:, 512:HID], inp2[:, 512:HID], o1k[t][:, 512:HID], OP.mult
                    )
                    nc.sync.dma_start(out_d[rows, 2 * HID : 3 * HID], out3[:])

                # ============ epilogue: output_two ============
                ep_cm = tc.tile_pool(name="ep", bufs=1)
                ep = ep_cm.__enter__()
                g1 = ep.tile([P, 1], f32, tag="g1")
                nc.vector.reduce_sum(g1[:], G[:], axis=AX)
                g1r = ep.tile([P, 1], f32r, tag="g1r")
                nc.vector.tensor_copy(g1r[:], g1[:])
                ones2_f = ep.tile([P, 2], f32, tag="ones2f")
                nc.gpsimd.memset(ones2_f[:], 1.0)
                ones2_r = ep.tile([P, 2], f32r, tag="ones2r")
                nc.vector.tensor_copy(ones2_r[:], ones2_f[:])
                zps = psA.tile([1, 2], f32, tag="A")
                nc.tensor.matmul(zps[:], g1r[:], ones2_r[:], start=True, stop=True)
                rcz = ep.tile([1, 1], f32, tag="rcz")
                nc.vector.reciprocal(rcz[:], zps[0:1, 0:1])
                # v^T [128, KC]
                vT = ep.tile([P, KC], f32r, tag="vT")
                nc.vector.tensor_tensor(vtb_f[:], vtb_f[:], vTa_f[:], OP.add)
                nc.vector.tensor_copy(vT[:], vtb_f[:])
                ua = psA.tile([1, 512], f32, tag="A")
                ub = psA.tile([1, 512], f32, tag="A")
                for k in range(KC):
                    nc.tensor.matmul(
                        ua[:], vT[:, k : k + 1], w2i[k][:, 0:512],
                        start=(k == 0), stop=(k == KC - 1),
                    )
                    nc.tensor.matmul(
                        ub[:], vT[:, k : k + 1], w2i[k][:, 512:HID],
                        start=(k == 0), stop=(k == KC - 1),
                    )
                o2row_r = ep.tile([1, HID], f32r, tag="o2rowr")
                nc.vector.tensor_scalar_mul(o2row_r[:, 0:512], ua[:], rcz[:])
                nc.vector.tensor_scalar_mul(o2row_r[:, 512:HID], ub[:], rcz[:])
                nc.vector.tensor_tensor(o2row_r[:], o2row_r[:], b2i_row[:], OP.add)
                # broadcast across partitions via K=1 matmuls (PE is idle here)
                o2bc = ep.tile([P, HID], f32, tag="o2bc")
                ob1 = psA.tile([P, 512], f32, tag="A")
                nc.tensor.matmul(ob1[:], ones_r[:], o2row_r[:, 0:512], start=True, stop=True)
                nc.scalar.copy(o2bc[:, 0:512], ob1[:])
                ob2 = psA.tile([P, 512], f32, tag="A")
                nc.tensor.matmul(ob2[:], ones_r[:], o2row_r[:, 512:HID], start=True, stop=True)
                nc.scalar.copy(o2bc[:, 512:HID], ob2[:])
                # out4 = o2 * O1 -> out[:, 3072:4096]
                for t in range(NT):
                    rows = slice(t * P, (t + 1) * P)
                    out4 = (wk if t % 2 == 0 else wk3).tile(
                        [P, HID], f32, tag=("out3" if t % 2 == 0 else "xt"),
                        name=f"out4_{t}",
                    )
                    nc.vector.tensor_tensor(
                        out4[:, 0:512], o1k[t][:, 0:512], o2bc[:, 0:512], OP.mult
                    )
                    eng4 = nc.gpsimd if t % 2 == 1 else nc.vector
                    eng4.tensor_tensor(
                        out4[:, 512:HID], o1k[t][:, 512:HID], o2bc[:, 512:HID], OP.mult
                    )
                    nc.sync.dma_start(out_d[rows, 3 * HID : 4 * HID], out4[:])
                ep_cm.__exit__(None, None, None)

    nc.compile()
    return nc


def _get_nc():
    global _NC
    if _NC is None:
        _NC = _build()
    return _NC


def kernel(input, memory, mask, in1_w, mem1_w, in2_w, in2_b, mem2_w, mem2_b):
    nc = _get_nc()

    def f(a):
        return np.ascontiguousarray(np.asarray(a, dtype=np.float32))

    inp = f(input)
    memv = f(memory)
    maskv = f(mask)
    i1 = f(in1_w)
    m1 = f(mem1_w)
    wi = f(in2_w)
    bi = f(in2_b).reshape(1, HID)
    wm = f(mem2_w)
    bm = f(mem2_b).reshape(1, HID)

    in_maps = []
    for i in range(NCORES):
        in_maps.append(
            {
                "x": inp[i],
                "mem": memv[i],
                "mask": maskv[i].reshape(1, LM),
                "i1w": i1,
                "m1w": m1,
                "i2w": wi,
                "i2b": bi,
                "m2w": wm,
                "m2b": bm,
            }
        )

    kwargs = {}
    if TRACE:
        kwargs["trace"] = True
    res = run_bass_kernel_spmd(nc, in_maps, core_ids=list(range(NCORES)), **kwargs)
    global LAST_RESULT
    LAST_RESULT = res

    out = np.stack([r["out"] for r in res.results])
    mem2 = np.stack([r["mem2o"] for r in res.results])
    return out, mem2
